# revision 13
# baseline (speedup 1.0000x reference)
"""Trainium2 Bass kernel for ExtractorLoss (PSD SNR loss).

loss = -mean_b( 10*log10( (mean wanted psd) / (mean unwanted psd) ) )
with psd[b,g] = (x @ cos_g)^2 + (x @ sin_g)^2 over a 201-bin frequency grid.

Math: grid frequencies are g/1800 cycles/sample (g = grid_bpm in 40..240,
fs = 30 Hz), so the DFT basis has period 1800 over t and half-period sign
symmetry: cos(2*pi*g*(tau+900j)/1800) = (-1)^{gj} cos(2*pi*g*tau/1800).
Folding the four 900-sample segments of x on host (plain sum for even g,
alternating sum for odd g) shrinks the GEMM contraction from 3600 to 900
with two parity classes — 4x less PE work and x DMA than the naive GEMM.

Sharding: data-parallel over batch across 8 NeuronCores (512 rows each).
Host packs, per core, a [900, 1428] bf16 tensor: [xeT(512) | xoT(512) |
basis_e(202) | basis_o(202)] so each contraction k-tile is one large DMA.
Odd-parity bins (100) are zero-padded to 101 so even/odd blocks align.
On device each of the 4 row-tiles accumulates both parity GEMMs into one
PSUM bank [128, 404] over 8 k-tiles; the epilogue squares cos/sin blocks
on ScalarE (with per-row accumulate), forms psd and the wanted-band
masked sum on DVE, and writes per-row (total, wanted) sums. The final
log/mean runs on host in float64.
"""

import functools
import sys

import numpy as np
import ml_dtypes

if "/opt/trn_rl_repo" not in sys.path:
    sys.path.insert(0, "/opt/trn_rl_repo")

# Problem constants (fixed by the problem spec).
B, T, NG = 4096, 3600, 201
NCORES = 8
BS = B // NCORES          # 512 batch rows per core
MT = BS // 128            # 4 output partition tiles per core
TF = T // 4               # 900 folded contraction length
KT = 128
NK = (TF + KT - 1) // KT  # 8 k-tiles (7x128 + 1x4)
NGE = 101                 # even-parity bins (grid 40,42..240)
NGO = 100                 # odd-parity bins (grid 41,43..239)
NGP = 101                 # odd padded to 101
NBH = 2 * NGP             # 202 basis cols per parity (cos|sin)
PSW = 2 * NBH             # 404 psum cols per row-tile
XCOL = 2 * BS             # 1024 x cols (xe | xo)
PCOL = XCOL + 2 * NBH     # 1428 packed cols per k row

BF16 = ml_dtypes.bfloat16


@functools.lru_cache(maxsize=1)
def _build_program():
    import concourse.bacc as bacc
    import concourse.mybir as mybir
    from concourse.bass import ts
    from concourse.tile import TileContext

    f32 = mybir.dt.float32
    bf16 = mybir.dt.bfloat16

    # Bacc (not raw Bass): its finalize() legalizes multi-wait instructions
    # into event-semaphore chains — walrus allows only 1 wait per inst.
    nc = bacc.Bacc()
    xb = nc.declare_dram_parameter("xb", [TF, PCOL], bf16, isOutput=False)
    mask = nc.declare_dram_parameter("mask", [BS, NBH // 2 * 2], f32, isOutput=False)
    out = nc.declare_dram_parameter("out", [BS, 2], f32, isOutput=True)

    ksizes = []
    off = 0
    while off < TF:
        sz = min(KT, TF - off)
        ksizes.append((off, sz))
        off += sz

    with TileContext(nc) as tc:
        with (
            tc.tile_pool(name="xk", bufs=NK) as xpool,
            tc.tile_pool(name="masks", bufs=1) as mpool,
            tc.tile_pool(name="psum", bufs=1, space="PSUM") as pspool,
            tc.tile_pool(name="epi", bufs=4) as epool,
            tc.tile_pool(name="outs", bufs=1) as opool,
        ):
            # Masks: one DMA, then bounce through DVE so epilogue DVE ops
            # need no DMA sync wait (HW instructions carry only one wait).
            mask_raw = mpool.tile([128, MT, NBH], f32, name="mask_raw")
            nc.sync.dma_start(
                out=mask_raw, in_=mask.rearrange("(m p) g -> p m g", p=128)
            )
            mask_sb = mpool.tile([128, MT, NBH], f32, name="mask_sb")
            nc.vector.tensor_copy(out=mask_sb, in_=mask_raw)

            # Two PSUM banks per row-tile: bank 0 = even parity, bank 1 = odd.
            # (Accumulation groups can't share a 2KB bank zero-region.)
            psums = [
                pspool.tile([128, 2, 512], f32, name=f"ps{m}", tag=f"ps{m}")
                for m in range(MT)
            ]

            for k, (off, sz) in enumerate(ksizes):
                xk = xpool.tile([128, PCOL], bf16, name="xk")
                nc.sync.dma_start(out=xk[:sz, :], in_=xb[off : off + sz, :])
                start, stop = (k == 0), (k == NK - 1)
                for m in range(MT):
                    # even parity: out -> psum bank 0
                    nc.tensor.matmul(
                        psums[m][:, 0, 0:NBH],
                        lhsT=xk[:sz, ts(m, 128)],
                        rhs=xk[:sz, XCOL : XCOL + NBH],
                        start=start,
                        stop=stop,
                    )
                    # odd parity: out -> psum bank 1
                    nc.tensor.matmul(
                        psums[m][:, 1, 0:NBH],
                        lhsT=xk[:sz, BS + m * 128 : BS + (m + 1) * 128],
                        rhs=xk[:sz, XCOL + NBH : XCOL + 2 * NBH],
                        start=start,
                        stop=stop,
                    )

            out_all = opool.tile([128, MT, 2], f32, name="out_all")
            for m in range(MT):
                ps3 = psums[m]
                cosv = ps3[:, :, 0:NGP]        # [128, 2, 101] cosE | cosO
                sinv = ps3[:, :, NGP:NBH]      # [128, 2, 101] sinE | sinO
                a2 = epool.tile([128, NBH], f32, tag="a2", name="a2")
                b2 = epool.tile([128, NBH], f32, tag="b2", name="b2")
                psd = epool.tile([128, NBH], f32, tag="psd", name="psd")
                mw = epool.tile([128, NBH], f32, tag="mw", name="mw")
                acc_a = epool.tile([128, 1], f32, tag="acca", name="acca")
                acc_b = epool.tile([128, 1], f32, tag="accb", name="accb")

                a2v = a2.rearrange("p (two x) -> p two x", two=2)
                b2v = b2.rearrange("p (two x) -> p two x", two=2)
                # Squares + per-row sums on ScalarE (single PSUM input each).
                nc.scalar.activation(
                    a2v, cosv, mybir.ActivationFunctionType.Square,
                    accum_out=acc_a,
                )
                nc.scalar.activation(
                    b2v, sinv, mybir.ActivationFunctionType.Square,
                    accum_out=acc_b,
                )
                # total = sum a^2 + sum b^2 (over all bins; pads are zero)
                nc.vector.tensor_add(out_all[:, m, 0:1], acc_a, acc_b)
                # wanted = sum mask * (a^2 + b^2)
                nc.vector.tensor_add(psd, a2, b2)
                nc.vector.tensor_mul(mw, psd, mask_sb[:, m, :])
                nc.vector.tensor_reduce(
                    out_all[:, m, 1:2], mw, axis=mybir.AxisListType.X,
                    op=mybir.AluOpType.add,
                )
            nc.gpsimd.dma_start(
                out=out.rearrange("(m p) c -> p m c", p=128), in_=out_all
            )

    # Run Bacc's compile passes (register allocation, 1-wait legalization
    # via event-semaphore chains) — the PJRT exec path doesn't finalize.
    nc.finalize()
    return nc


def _host_prep(x, f_true_bpm, fs, delta_bpm, sampling_bpm, fmin_bpm, fmax_bpm):
    fs = int(fs)
    delta = int(delta_bpm)
    samp = int(sampling_bpm)
    fmin = int(fmin_bpm)
    fmax = int(fmax_bpm)

    n_grid = (fmax - fmin) // samp + 1
    assert n_grid == NG and fs == 30 and samp == 1, (n_grid, fs, samp)
    grid_bpm = fmin + samp * np.arange(n_grid, dtype=np.int64)
    ge = grid_bpm[grid_bpm % 2 == 0]  # 101 even bins
    go = grid_bpm[grid_bpm % 2 == 1]  # 100 odd bins

    # Folded basis over tau in [0, 900): theta = 2*pi*g*tau/1800.
    tau = np.arange(TF, dtype=np.float64)
    thE = 2.0 * np.pi * ge[:, None] * tau[None, :] / 1800.0  # [101, 900]
    thO = 2.0 * np.pi * go[:, None] * tau[None, :] / 1800.0  # [100, 900]
    basis = np.zeros((TF, 2 * NBH), dtype=BF16)
    basis[:, 0:NGE] = np.cos(thE).T.astype(BF16)
    basis[:, NGP : NGP + NGE] = np.sin(thE).T.astype(BF16)
    basis[:, NBH : NBH + NGO] = np.cos(thO).T.astype(BF16)
    basis[:, NBH + NGP : NBH + NGP + NGO] = np.sin(thO).T.astype(BF16)

    # Fold x: 4 segments of 900; even g sums plain, odd g alternates.
    s = x.astype(np.float64).reshape(B, 4, TF)
    xe = (s[:, 0] + s[:, 1] + s[:, 2] + s[:, 3]).astype(BF16)  # [B, 900]
    xo = (s[:, 0] - s[:, 1] + s[:, 2] - s[:, 3]).astype(BF16)

    # Wanted-band masks in [maskE(101) | maskO(100) pad] layout.
    f64 = f_true_bpm.astype(np.int64)
    mask = np.zeros((B, NBH), dtype=np.float32)
    mask[:, 0:NGE] = np.abs(ge[None, :] - f64[:, None]) <= delta
    mask[:, NGP : NGP + NGO] = np.abs(go[None, :] - f64[:, None]) <= delta

    in_maps = []
    for c in range(NCORES):
        sl = slice(c * BS, (c + 1) * BS)
        xbp = np.empty((TF, PCOL), dtype=BF16)
        xbp[:, 0:BS] = xe[sl].T
        xbp[:, BS:XCOL] = xo[sl].T
        xbp[:, XCOL:] = basis
        in_maps.append(
            {"xb": xbp, "mask": np.ascontiguousarray(mask[sl])}
        )

    n_wanted = 2 * delta // samp + 1
    n_unwanted = n_grid - n_wanted
    return in_maps, n_wanted, n_unwanted


def _finalize(outs, n_wanted, n_unwanted):
    # outs: list of [BS, 2] fp32 per core with (total_sum, wanted_sum) rows.
    full = np.concatenate(outs, axis=0).astype(np.float64)  # [B, 2]
    total, wanted = full[:, 0], full[:, 1]
    term1 = wanted / n_wanted
    term2 = (total - wanted) / n_unwanted
    snr = 10.0 * np.log10(term1 / term2)
    return np.array(-snr.mean(), dtype=np.float32)


def kernel(x, f_true_bpm, fs, delta_bpm, sampling_bpm, fmin_bpm, fmax_bpm):
    from concourse.bass_utils import run_bass_kernel_spmd

    x = np.asarray(x, dtype=np.float32)
    f_true_bpm = np.asarray(f_true_bpm)
    in_maps, n_wanted, n_unwanted = _host_prep(
        x, f_true_bpm, fs, delta_bpm, sampling_bpm, fmin_bpm, fmax_bpm
    )
    nc = _build_program()
    res = run_bass_kernel_spmd(nc, in_maps, core_ids=list(range(NCORES)))
    outs = [r["out"] for r in res.results]
    return _finalize(outs, n_wanted, n_unwanted)


# revision 14
# speedup vs baseline: 1.1286x; 1.1286x over previous
"""Trainium2 Bass kernel for ExtractorLoss (PSD SNR loss).

loss = -mean_b( 10*log10( (mean wanted psd) / (mean unwanted psd) ) )
with psd[b,g] = (x @ cos_g)^2 + (x @ sin_g)^2 over a 201-bin frequency grid.

Math: grid frequencies are g/1800 cycles/sample (g = grid_bpm in 40..240,
fs = 30 Hz), so the DFT basis has period 1800 over t and half-period sign
symmetry: cos(2*pi*g*(tau+900j)/1800) = (-1)^{gj} cos(2*pi*g*tau/1800).
Folding the four 900-sample segments of x on host (plain sum for even g,
alternating sum for odd g) shrinks the GEMM contraction from 3600 to 900
with two parity classes — 4x less PE work and x DMA than the naive GEMM.

Sharding: data-parallel over batch across 8 NeuronCores (512 rows each).
Host packs, per core, a [900, 1428] bf16 tensor: [xeT(512) | xoT(512) |
basis_e(202) | basis_o(202)] so each contraction k-tile is one large DMA.
Odd-parity bins (100) are zero-padded to 101 so even/odd blocks align.
On device each of the 4 row-tiles accumulates both parity GEMMs into one
PSUM bank [128, 404] over 8 k-tiles; the epilogue squares cos/sin blocks
on ScalarE (with per-row accumulate), forms psd and the wanted-band
masked sum on DVE, and writes per-row (total, wanted) sums. The final
log/mean runs on host in float64.
"""

import functools
import sys

import numpy as np
import ml_dtypes

if "/opt/trn_rl_repo" not in sys.path:
    sys.path.insert(0, "/opt/trn_rl_repo")

# Problem constants (fixed by the problem spec).
B, T, NG = 4096, 3600, 201
NCORES = 8
BS = B // NCORES          # 512 batch rows per core
MT = BS // 128            # 4 output partition tiles per core
TF = T // 4               # 900 folded contraction length
KT = 128
NK = (TF + KT - 1) // KT  # 8 k-tiles (7x128 + 1x4)
NGE = 101                 # even-parity bins (grid 40,42..240)
NGO = 100                 # odd-parity bins (grid 41,43..239)
NGP = 101                 # odd padded to 101
NBH = 2 * NGP             # 202 basis cols per parity (cos|sin)
PSW = 2 * NBH             # 404 psum cols per row-tile
XCOL = 2 * BS             # 1024 x cols (xe | xo)
PCOL = XCOL + 2 * NBH     # 1428 packed cols per k row

BF16 = ml_dtypes.bfloat16


@functools.lru_cache(maxsize=1)
def _build_program():
    import concourse.bacc as bacc
    import concourse.mybir as mybir
    from concourse.bass import ts
    from concourse.tile import TileContext

    f32 = mybir.dt.float32
    bf16 = mybir.dt.bfloat16

    # Bacc (not raw Bass): its finalize() legalizes multi-wait instructions
    # into event-semaphore chains — walrus allows only 1 wait per inst.
    nc = bacc.Bacc()
    xb = nc.declare_dram_parameter("xb", [TF, PCOL], bf16, isOutput=False)
    mask = nc.declare_dram_parameter("mask", [BS, NBH], bf16, isOutput=False)
    out = nc.declare_dram_parameter("out", [BS, 2], f32, isOutput=True)

    ksizes = []
    off = 0
    while off < TF:
        sz = min(KT, TF - off)
        ksizes.append((off, sz))
        off += sz

    with TileContext(nc) as tc:
        with (
            tc.tile_pool(name="xk", bufs=NK) as xpool,
            tc.tile_pool(name="masks", bufs=1) as mpool,
            tc.tile_pool(name="psum", bufs=1, space="PSUM") as pspool,
            tc.tile_pool(name="epi", bufs=4) as epool,
            tc.tile_pool(name="outs", bufs=1) as opool,
        ):
            # One PSUM tile spanning all 8 banks: [m(4), parity(2), bank(512)].
            # (Each matmul accumulation group owns one 2KB bank zero-region.)
            ps = pspool.tile([128, MT, 2, 512], f32, name="ps")

            for k, (off, sz) in enumerate(ksizes):
                xk = xpool.tile([128, PCOL], bf16, name="xk")
                nc.sync.dma_start(out=xk[:sz, :], in_=xb[off : off + sz, :])
                start, stop = (k == 0), (k == NK - 1)
                for m in range(MT):
                    # even parity: out -> psum bank 2m
                    nc.tensor.matmul(
                        ps[:, m, 0, 0:NBH],
                        lhsT=xk[:sz, ts(m, 128)],
                        rhs=xk[:sz, XCOL : XCOL + NBH],
                        start=start,
                        stop=stop,
                    )
                    # odd parity: out -> psum bank 2m+1
                    nc.tensor.matmul(
                        ps[:, m, 1, 0:NBH],
                        lhsT=xk[:sz, BS + m * 128 : BS + (m + 1) * 128],
                        rhs=xk[:sz, XCOL + NBH : XCOL + 2 * NBH],
                        start=start,
                        stop=stop,
                    )

            # Mask load last (only the epilogue needs it).
            mask_sb = mpool.tile([128, MT, NBH], bf16, name="mask_sb")
            nc.sync.dma_start(
                out=mask_sb, in_=mask.rearrange("(m p) g -> p m g", p=128)
            )

            # Whole-core epilogue: 2 ACT squares + 4 DVE ops over all 808
            # retained psum columns at once (per-op overhead amortized).
            cosv = ps[:, :, :, 0:NGP]      # [128, 4, 2, 101] cos blocks
            sinv = ps[:, :, :, NGP:NBH]    # [128, 4, 2, 101] sin blocks
            a2 = epool.tile([128, MT, 2, NGP], f32, name="a2")
            b2 = epool.tile([128, MT, 2, NGP], f32, name="b2")
            psd = epool.tile([128, MT, NBH], f32, name="psd")
            mw = epool.tile([128, MT, NBH], f32, name="mw")
            out_all = opool.tile([128, MT, 2], f32, name="out_all")

            nc.scalar.activation(a2, cosv, mybir.ActivationFunctionType.Square)
            nc.scalar.activation(b2, sinv, mybir.ActivationFunctionType.Square)
            psd_v = psd.rearrange("p m (two x) -> p m two x", two=2)
            nc.vector.tensor_add(psd_v, a2, b2)
            # per-row-tile totals and wanted-band sums
            nc.vector.tensor_reduce(
                out_all[:, :, 0], psd, axis=mybir.AxisListType.X,
                op=mybir.AluOpType.add,
            )
            nc.vector.tensor_mul(mw, psd, mask_sb)
            nc.vector.tensor_reduce(
                out_all[:, :, 1], mw, axis=mybir.AxisListType.X,
                op=mybir.AluOpType.add,
            )
            nc.sync.dma_start(
                out=out.rearrange("(m p) c -> p m c", p=128), in_=out_all
            )

    # Run Bacc's compile passes (register allocation, 1-wait legalization
    # via event-semaphore chains) — the PJRT exec path doesn't finalize.
    nc.finalize()
    return nc


def _host_prep(x, f_true_bpm, fs, delta_bpm, sampling_bpm, fmin_bpm, fmax_bpm):
    fs = int(fs)
    delta = int(delta_bpm)
    samp = int(sampling_bpm)
    fmin = int(fmin_bpm)
    fmax = int(fmax_bpm)

    n_grid = (fmax - fmin) // samp + 1
    assert n_grid == NG and fs == 30 and samp == 1, (n_grid, fs, samp)
    grid_bpm = fmin + samp * np.arange(n_grid, dtype=np.int64)
    ge = grid_bpm[grid_bpm % 2 == 0]  # 101 even bins
    go = grid_bpm[grid_bpm % 2 == 1]  # 100 odd bins

    # Folded basis over tau in [0, 900): theta = 2*pi*g*tau/1800.
    tau = np.arange(TF, dtype=np.float64)
    thE = 2.0 * np.pi * ge[:, None] * tau[None, :] / 1800.0  # [101, 900]
    thO = 2.0 * np.pi * go[:, None] * tau[None, :] / 1800.0  # [100, 900]
    basis = np.zeros((TF, 2 * NBH), dtype=BF16)
    basis[:, 0:NGE] = np.cos(thE).T.astype(BF16)
    basis[:, NGP : NGP + NGE] = np.sin(thE).T.astype(BF16)
    basis[:, NBH : NBH + NGO] = np.cos(thO).T.astype(BF16)
    basis[:, NBH + NGP : NBH + NGP + NGO] = np.sin(thO).T.astype(BF16)

    # Fold x: 4 segments of 900; even g sums plain, odd g alternates.
    s = x.astype(np.float64).reshape(B, 4, TF)
    xe = (s[:, 0] + s[:, 1] + s[:, 2] + s[:, 3]).astype(BF16)  # [B, 900]
    xo = (s[:, 0] - s[:, 1] + s[:, 2] - s[:, 3]).astype(BF16)

    # Wanted-band masks in [maskE(101) | maskO(100) pad] layout.
    f64 = f_true_bpm.astype(np.int64)
    mask = np.zeros((B, NBH), dtype=BF16)
    mask[:, 0:NGE] = np.abs(ge[None, :] - f64[:, None]) <= delta
    mask[:, NGP : NGP + NGO] = np.abs(go[None, :] - f64[:, None]) <= delta

    in_maps = []
    for c in range(NCORES):
        sl = slice(c * BS, (c + 1) * BS)
        xbp = np.empty((TF, PCOL), dtype=BF16)
        xbp[:, 0:BS] = xe[sl].T
        xbp[:, BS:XCOL] = xo[sl].T
        xbp[:, XCOL:] = basis
        in_maps.append(
            {"xb": xbp, "mask": np.ascontiguousarray(mask[sl])}
        )

    n_wanted = 2 * delta // samp + 1
    n_unwanted = n_grid - n_wanted
    return in_maps, n_wanted, n_unwanted


def _finalize(outs, n_wanted, n_unwanted):
    # outs: list of [BS, 2] fp32 per core with (total_sum, wanted_sum) rows.
    full = np.concatenate(outs, axis=0).astype(np.float64)  # [B, 2]
    total, wanted = full[:, 0], full[:, 1]
    term1 = wanted / n_wanted
    term2 = (total - wanted) / n_unwanted
    snr = 10.0 * np.log10(term1 / term2)
    return np.array(-snr.mean(), dtype=np.float32)


def kernel(x, f_true_bpm, fs, delta_bpm, sampling_bpm, fmin_bpm, fmax_bpm):
    from concourse.bass_utils import run_bass_kernel_spmd

    x = np.asarray(x, dtype=np.float32)
    f_true_bpm = np.asarray(f_true_bpm)
    in_maps, n_wanted, n_unwanted = _host_prep(
        x, f_true_bpm, fs, delta_bpm, sampling_bpm, fmin_bpm, fmax_bpm
    )
    nc = _build_program()
    res = run_bass_kernel_spmd(nc, in_maps, core_ids=list(range(NCORES)))
    outs = [r["out"] for r in res.results]
    return _finalize(outs, n_wanted, n_unwanted)
